# revision 5
# baseline (speedup 1.0000x reference)
"""Trainium2 Bass kernel for nn_AffineExpert (diag + rank-R linear recurrence).

Math: s_{t+1} = a_t*s_t + u_t + U (g_t * (V^T s_t)),  s_0 = 0, output s_S.
  a = sigmoid(x@Wa^T + ba), g = x@Wg^T + bg, u = x@Wu^T + bu.

Strategy per core (data-parallel over batch, 2 rows/core on 8 cores):
  * big projections as fp16 PE matmuls (fp32 PSUM accum), N=512 tiles,
    output layout [h-partition, t-free],
  * the recurrence is linear in the rank-R channel q_t = g_t*(V^T s_t);
    with q known, s is a pure diagonal-decay scan, done by the DVE
    tensor_tensor_scan instruction along t,
  * q itself is found by a fixed-point iteration (loop gain ~3%): per
    512-step chunk, scan -> project (V^T) -> q-update, 2 correction
    iterations + final scan give ~3e-4 relative error overall,
  * chunks chain exactly through the carried state column; chunk c+1
    projections (PE) overlap chunk c iterations (DVE) via Tile scheduling.
"""
import numpy as np

import concourse.bass as bass
import concourse.mybir as mybir
import concourse.tile as tile
from concourse import bacc
from concourse.bass_utils import run_bass_kernel_spmd

f32 = mybir.dt.float32
f16 = mybir.dt.float16
AF = mybir.ActivationFunctionType
OP = mybir.AluOpType

B, S, D, H, R = 16, 2048, 1024, 1024, 16
N_CORES = 8
B_CORE = B // N_CORES
CHUNK = 512
K_INNER = 3  # scans per chunk = K_INNER + 1 (z0 + (K_INNER-1) corr + final)


def build_kernel(B_core=B_CORE, S_=S, D_=D, H_=H, R_=R, C=CHUNK, k_inner=K_INNER):
    KC, HC, NCH = D_ // 128, H_ // 128, S_ // C
    # Bacc (not raw Bass): its compile() pass legalizes semaphore waits
    # (1-wait-per-instruction hardware limit) via event semaphores.
    nc = bacc.Bacc("TRN2")

    xT = nc.dram_tensor("xT", [B_core, KC, 128, S_], f32, kind="ExternalInput")
    waT = nc.dram_tensor("waT", [KC, 128, H_], f32, kind="ExternalInput")
    wuT = nc.dram_tensor("wuT", [KC, 128, H_], f32, kind="ExternalInput")
    wgT = nc.dram_tensor("wgT", [KC, 128, R_], f32, kind="ExternalInput")
    uT_d = nc.dram_tensor("uT", [R_, H_], f32, kind="ExternalInput")
    v_d = nc.dram_tensor("v", [H_, R_], f32, kind="ExternalInput")
    ba_d = nc.dram_tensor("ba", [128, HC], f32, kind="ExternalInput")
    bu_d = nc.dram_tensor("bu", [128, HC], f32, kind="ExternalInput")
    bg_d = nc.dram_tensor("bg", [R_, 1], f32, kind="ExternalInput")
    out_d = nc.dram_tensor("out", [B_core, H_], f32, kind="ExternalOutput")

    with tile.TileContext(nc) as tc:
        with tc.tile_pool(name="persist", bufs=1) as persist, \
             tc.tile_pool(name="work", bufs=2) as work, \
             tc.tile_pool(name="trans", bufs=3) as trans, \
             tc.tile_pool(name="ps_big", bufs=4, space="PSUM") as ps_big, \
             tc.tile_pool(name="ps_p", bufs=2, space="PSUM") as ps_p, \
             tc.tile_pool(name="ps_small", bufs=2, space="PSUM") as ps_small:

            # ---------- startup: stage weights to fp16, init state ----------
            w16a = persist.tile([128, KC, H_], f16)
            w16u = persist.tile([128, KC, H_], f16)
            wg16 = persist.tile([128, KC, R_], f16)
            v16 = persist.tile([128, HC, R_], f16)
            u16T = persist.tile([R_, H_], f16)
            ba_t = persist.tile([128, HC], f32)
            bu_t = persist.tile([128, HC], f32)
            bg_t = persist.tile([R_, 1], f32)
            state_cols = persist.tile([128, B_core * HC], f32)
            p_first = persist.tile([R_, B_core], f32)

            # gpsimd SWDGE casts fp32->fp16 in transit; no landing tiles, so
            # every staging DMA carries at most one wait.
            for kc in range(KC):
                nc.gpsimd.dma_start(w16a[:, kc, :], waT[kc])
                nc.gpsimd.dma_start(w16u[:, kc, :], wuT[kc])
                nc.gpsimd.dma_start(wg16[:, kc, :], wgT[kc])
            nc.gpsimd.dma_start(
                v16[:], v_d.rearrange("(hc p) r -> p hc r", p=128))
            nc.gpsimd.dma_start(u16T[:], uT_d[:, :])
            nc.sync.dma_start(ba_t[:], ba_d[:, :])
            nc.sync.dma_start(bu_t[:], bu_d[:, :])
            nc.sync.dma_start(bg_t[:], bg_d[:, :])

            nc.vector.memset(state_cols[:], 0.0)
            nc.vector.memset(p_first[:], 0.0)

            # ---------- chunk loop ----------
            for c in range(NCH):
                t0 = c * C
                x16 = {}
                for row in range(B_core):
                    for kc in range(KC):
                        xt = work.tile([128, C], f16, tag=f"x_{row}_{kc}")
                        nc.gpsimd.dma_start(
                            xt[:], xT[row, kc, :, t0:t0 + C])
                        x16[row, kc] = xt

                a_tiles = {}
                g_tiles = {}
                zl_tiles = {}
                p0s_tiles = {}
                for row in range(B_core):
                    # g projection [R, C]
                    gp = ps_small.tile([R_, C], f32, tag="small")
                    for kc in range(KC):
                        nc.tensor.matmul(
                            gp[:], wg16[:, kc, :], x16[row, kc][:],
                            start=(kc == 0), stop=(kc == KC - 1))
                    gt = work.tile([R_, C], f32, tag=f"g_{row}")
                    nc.scalar.activation(gt[:], gp[:], AF.Identity, bias=bg_t[:])
                    g_tiles[row] = gt

                    zlast = work.tile([128, HC], f32, tag=f"zl_{row}")
                    zl_tiles[row] = zlast
                    p0p = ps_p.tile([R_, C], f32, tag="pp")
                    for hc in range(HC):
                        hs = slice(hc * 128, (hc + 1) * 128)
                        ap = ps_big.tile([128, C], f32, tag="big")
                        for kc in range(KC):
                            nc.tensor.matmul(
                                ap[:], w16a[:, kc, hs], x16[row, kc][:],
                                start=(kc == 0), stop=(kc == KC - 1))
                        at = work.tile([128, C], f32, tag=f"a_{row}_{hc}")
                        nc.scalar.activation(
                            at[:], ap[:], AF.Sigmoid, bias=ba_t[:, hc:hc + 1])
                        a_tiles[row, hc] = at

                        up = ps_big.tile([128, C], f32, tag="big")
                        for kc in range(KC):
                            nc.tensor.matmul(
                                up[:], w16u[:, kc, hs], x16[row, kc][:],
                                start=(kc == 0), stop=(kc == KC - 1))
                        ut = trans.tile([128, C], f32, tag="ut")
                        nc.scalar.activation(
                            ut[:], up[:], AF.Identity, bias=bu_t[:, hc:hc + 1])

                        # z0 scan: state column as initial
                        col = row * HC + hc
                        z0 = trans.tile([128, C], f32, tag="z0")
                        nc.vector.tensor_tensor_scan(
                            z0[:], at[:], ut[:],
                            state_cols[:, col:col + 1], OP.mult, OP.add)
                        nc.vector.tensor_copy(
                            zlast[:, col - row * HC:col - row * HC + 1],
                            z0[:, C - 1:C])
                        z16 = trans.tile([128, C], f16, tag="z16")
                        nc.vector.tensor_copy(z16[:], z0[:])
                        nc.tensor.matmul(
                            p0p[:], v16[:, hc, :], z16[:],
                            start=(hc == 0), stop=(hc == HC - 1))

                    p0s = work.tile([R_, C], f32, tag=f"p0s_{row}")
                    nc.vector.tensor_copy(p0s[:], p0p[:])
                    p0s_tiles[row] = p0s

                # ---------- fixed-point iterations ----------
                q16 = {}
                for row in range(B_core):
                    pf = work.tile([R_, C], f32, tag=f"pf_{row}")
                    nc.vector.tensor_copy(
                        pf[:, 0:1], p_first[:, row:row + 1])
                    nc.vector.tensor_copy(
                        pf[:, 1:C], p0s_tiles[row][:, 0:C - 1])
                    qt = work.tile([R_, C], f16, tag=f"q_{row}")
                    nc.vector.tensor_tensor(
                        qt[:], g_tiles[row][:], pf[:], OP.mult)
                    q16[row] = qt

                for j in range(1, k_inner):
                    for row in range(B_core):
                        ppj = ps_p.tile([R_, C], f32, tag="pp")
                        for hc in range(HC):
                            uqp = ps_big.tile([128, C], f32, tag="big")
                            nc.tensor.matmul(
                                uqp[:],
                                u16T[:, hc * 128:(hc + 1) * 128],
                                q16[row][:], start=True, stop=True)
                            c16 = trans.tile([128, C], f16, tag="c16")
                            nc.vector.tensor_tensor_scan(
                                c16[:], a_tiles[row, hc][:], uqp[:],
                                0.0, OP.mult, OP.add)
                            nc.tensor.matmul(
                                ppj[:], v16[:, hc, :], c16[:],
                                start=(hc == 0), stop=(hc == HC - 1))
                        pf = work.tile([R_, C], f32, tag=f"pf_{row}")
                        nc.vector.tensor_copy(
                            pf[:, 0:1], p_first[:, row:row + 1])
                        nc.vector.tensor_tensor(
                            pf[:, 1:C], p0s_tiles[row][:, 0:C - 1],
                            ppj[:, 0:C - 1], OP.add)
                        qt = work.tile([R_, C], f16, tag=f"q_{row}")
                        nc.vector.tensor_tensor(
                            qt[:], g_tiles[row][:], pf[:], OP.mult)
                        q16[row] = qt

                # ---------- final scan: state + next p_first ----------
                for row in range(B_core):
                    pfp = ps_small.tile([R_, C], f32, tag="small")
                    for hc in range(HC):
                        uqp = ps_big.tile([128, C], f32, tag="big")
                        nc.tensor.matmul(
                            uqp[:], u16T[:, hc * 128:(hc + 1) * 128],
                            q16[row][:], start=True, stop=True)
                        c16 = trans.tile([128, C], f16, tag="c16")
                        nc.vector.tensor_tensor_scan(
                            c16[:], a_tiles[row, hc][:], uqp[:],
                            0.0, OP.mult, OP.add)
                        col = row * HC + hc
                        nc.vector.tensor_tensor(
                            state_cols[:, col:col + 1],
                            zl_tiles[row][:, hc:hc + 1],
                            c16[:, C - 1:C], OP.add)
                        nc.tensor.matmul(
                            pfp[:, 0:1], v16[:, hc, :], c16[:, C - 1:C],
                            start=(hc == 0), stop=(hc == HC - 1))
                    nc.vector.tensor_tensor(
                        p_first[:, row:row + 1],
                        p0s_tiles[row][:, C - 1:C],
                        pfp[:, 0:1], OP.add)

            # ---------- output ----------
            for row in range(B_core):
                for hc in range(HC):
                    col = row * HC + hc
                    nc.sync.dma_start(
                        out_d[row, hc * 128:(hc + 1) * 128],
                        state_cols[:, col:col + 1])
    nc.finalize()
    return nc


def make_in_maps(x, Wa, ba, Wg, bg, Wu, bu, u, v, n_cores=N_CORES):
    """Shard + lay out host-side (pure layout transforms, fp32 kept)."""
    B_, S_, D_ = x.shape
    H_, R_ = u.shape
    KC, HC = D_ // 128, H_ // 128
    b_core = B_ // n_cores
    waT = np.ascontiguousarray(Wa.T).reshape(KC, 128, H_)
    wuT = np.ascontiguousarray(Wu.T).reshape(KC, 128, H_)
    wgT = np.ascontiguousarray(Wg.T).reshape(KC, 128, R_)
    uT = np.ascontiguousarray(u.T)
    ba_h = np.ascontiguousarray(ba.reshape(HC, 128).T)
    bu_h = np.ascontiguousarray(bu.reshape(HC, 128).T)
    bg_h = np.ascontiguousarray(bg.reshape(R_, 1))
    in_maps = []
    for core in range(n_cores):
        rows = slice(core * b_core, (core + 1) * b_core)
        xT = np.ascontiguousarray(
            x[rows].transpose(0, 2, 1)).reshape(b_core, KC, 128, S_)
        in_maps.append({
            "xT": xT, "waT": waT, "wuT": wuT, "wgT": wgT, "uT": uT,
            "v": np.ascontiguousarray(v), "ba": ba_h, "bu": bu_h, "bg": bg_h,
        })
    return in_maps


def kernel(x, Wa, ba, Wg, bg, Wu, bu, u, v):
    x = np.asarray(x, dtype=np.float32)
    in_maps = make_in_maps(
        x, np.asarray(Wa), np.asarray(ba), np.asarray(Wg), np.asarray(bg),
        np.asarray(Wu), np.asarray(bu), np.asarray(u), np.asarray(v))
    nc = build_kernel()
    res = run_bass_kernel_spmd(nc, in_maps, core_ids=list(range(N_CORES)))
    return np.concatenate(
        [res.results[i]["out"] for i in range(N_CORES)], axis=0)


if __name__ == "__main__":
    import reference  # only when run manually next to reference.py

    inputs = {k: np.asarray(v) for k, v in reference.setup_inputs().items()}
    got = kernel(**inputs)
    exp = np.asarray(reference.reference(**inputs))
    print("relmax:", np.abs(got - exp).max() / np.abs(exp).max())
